# revision 35
# baseline (speedup 1.0000x reference)
"""Trainium2 Bass kernel for nn_AttentionLayer: attention + residual LayerNorm.

Reference (per batch b):
    q/k/v = x @ W* + b*             [S, D], heads H=16 x HD=64
    attn  = softmax(q k^T / 8)      per head
    ctx   = attn @ v
    out   = LayerNorm(x + ctx) * gamma + beta
    am    = attn.mean(heads)

Sharding: 8 cores = 2 batches x 4 row-blocks. Core c handles batch c//4 and
query rows [512*(c%4), 512*(c%4+1)).  K/V are computed for the full batch on
every core (duplicated, no collectives); attention/softmax/mean/LN are fully
local to the core's 512 query rows.

Formulation: scores are computed TRANSPOSED (sk on partitions, sq on free),
softmax denominators via ones-matmul on the PE (broadcast-replicated across
partitions), so P~ = exp(scores^T) feeds the PV matmul directly from SBUF with
no transposes of the big attention matrix.
"""

import os

os.environ.pop("JAX_PLATFORMS", None)

import numpy as np

B, S, D, H, HD = 2, 2048, 1024, 16, 64
NCORES = 8
GROUPS = 4  # cores per batch
SQ = S // GROUPS  # 512 query rows per core
P = 128
DCH = D // P  # 8 d-chunks
SKC = S // P  # 16 sk-chunks
SQT = SQ // P  # 4 sq row blocks per core
EPS = 1e-5

_CACHE = {}


def _ensure_ntff_hook():
    """Restore the axon NTFF profiling hook if the image lacks
    antenv.axon_hooks (needed only when tracing; harmless otherwise)."""
    import sys
    import types

    try:
        from antenv.axon_hooks import get_axon_ntff_profile_hook  # noqa: F401

        return
    except ImportError:
        pass
    try:
        import antenv
        from trn_agent_boot.trn_boot import _ntff_profile_via_ctypes

        so = "/opt/axon/libaxon_pjrt.so"
        if not os.path.exists(so):
            return
        hook = _ntff_profile_via_ctypes(so)
        mod = types.ModuleType("antenv.axon_hooks")
        state = {"hook": hook}
        mod.set_axon_ntff_profile_hook = lambda h: state.__setitem__("hook", h)
        mod.get_axon_ntff_profile_hook = lambda: state["hook"]
        sys.modules["antenv.axon_hooks"] = mod
        antenv.axon_hooks = mod
    except Exception:
        pass


def _build_nc():
    from contextlib import ExitStack

    import concourse.bass as bass
    import concourse.mybir as mybir
    import concourse.tile as tile
    from concourse.masks import make_identity

    f32 = mybir.dt.float32
    bf16 = mybir.dt.bfloat16
    Alu = mybir.AluOpType
    Act = mybir.ActivationFunctionType

    nc = bass.Bass(target_bir_lowering=False, trn_type="TRN2")

    xfull = nc.dram_tensor("xfull", [S, D], f32, kind="ExternalInput")
    xq = nc.dram_tensor("xq", [SQ, D], f32, kind="ExternalInput")
    wq = nc.dram_tensor("wq", [D, D], f32, kind="ExternalInput")
    wk = nc.dram_tensor("wk", [D, D], f32, kind="ExternalInput")
    wv = nc.dram_tensor("wv", [D, D], f32, kind="ExternalInput")
    bq = nc.dram_tensor("bq", [D], f32, kind="ExternalInput")
    bk = nc.dram_tensor("bk", [D], f32, kind="ExternalInput")
    bv = nc.dram_tensor("bv", [D], f32, kind="ExternalInput")
    gamma = nc.dram_tensor("gamma", [D], f32, kind="ExternalInput")
    beta = nc.dram_tensor("beta", [D], f32, kind="ExternalInput")
    out_r = nc.dram_tensor("out_rows", [SQ, D], f32, kind="ExternalOutput")
    am_r = nc.dram_tensor("am_rows", [SQ, S], f32, kind="ExternalOutput")

    def bcast_part(ap, n):
        # replicate a [1, F]-ish AP across n partitions (DRAM source only)
        return bass.AP(ap.tensor, ap.offset, [[0, n]] + list(ap.ap))

    def free_bcast(ap, reps, inner):
        # [P, inner] -> [P, reps, inner] with 0-stride middle dim
        a = list(ap.ap)
        return bass.AP(ap.tensor, ap.offset, [a[0], [0, reps], a[-1]])

    with tile.TileContext(nc) as tc, ExitStack() as top:
        const = top.enter_context(tc.tile_pool(name="const", bufs=1))
        persist = top.enter_context(tc.tile_pool(name="persist", bufs=1))
        psc = top.enter_context(tc.tile_pool(name="psc", bufs=4, space="PSUM"))

        ident32 = const.tile([P, P], f32)
        make_identity(nc, ident32)
        identbf = const.tile([P, P], bf16)
        make_identity(nc, identbf)
        ones_mat = const.tile([P, P], bf16)
        nc.vector.memset(ones_mat, 1.0)
        ones_row = const.tile([1, SQ], bf16)
        nc.vector.memset(ones_row, 1.0)
        epst = const.tile([P, 1], f32)
        nc.vector.memset(epst, EPS)
        negln16 = const.tile([P, 1], f32)
        nc.vector.memset(negln16, -float(np.log(H)))
        # biases transposed onto partitions: [128, DCH] (d = j*128 + p) and
        # per-head [64, H] for the v bias
        bkTt = const.tile([P, DCH], f32)
        nc.gpsimd.dma_start(bkTt, bass.AP(bk, 0, [[1, P], [P, DCH]]))
        bqT8 = const.tile([P, DCH], f32)
        nc.gpsimd.dma_start(bqT8, bass.AP(bq, 0, [[1, P], [P, DCH]]))
        nc.vector.tensor_scalar_mul(bqT8, bqT8, 1.0 / float(np.sqrt(HD)))
        bvH = const.tile([64, H], f32)
        nc.gpsimd.dma_start(bvH, bass.AP(bv, 0, [[1, 64], [64, H]]))
        nc.vector.tensor_scalar_mul(bvH, bvH, 1.0 / H)
        # warm the PE clock gate with a short dense burst
        for _ in range(16):
            wps = psc.tile([P, SQ], f32, tag="sc")
            nc.tensor.matmul(
                wps, lhsT=ones_mat, rhs=free_bcast(ones_mat, 4, P),
                start=True, stop=True,
            )
        # biases as bf16 [1, D]
        bqf = const.tile([1, D], f32)
        nc.gpsimd.dma_start(bqf, bcast_part(bq.ap(), 1))
        bkf = const.tile([1, D], f32)
        nc.gpsimd.dma_start(bkf, bcast_part(bk.ap(), 1))
        bvf = const.tile([1, D], f32)
        nc.gpsimd.dma_start(bvf, bcast_part(bv.ap(), 1))
        bqb = const.tile([1, D], bf16)
        nc.vector.tensor_copy(bqb, bqf)
        bkb = const.tile([1, D], bf16)
        nc.vector.tensor_copy(bkb, bkf)
        bvb = const.tile([1, D], bf16)
        nc.vector.tensor_copy(bvb, bvf)

        # big persistent tensors (live to the end)
        kT = persist.tile([P, DCH, S], bf16)  # k^T: [dout, s]
        # v: [s, per-head (64 cols + a ones column)] chunked by s
        HD1 = HD + 1
        vN = persist.tile([P, SKC, H * HD1], bf16)
        qT = persist.tile([P, DCH, SQ], bf16)  # (q/8)^T for this core's rows
        nc.vector.memset(
            bass.AP(vN.tensor, vN.offset + HD,
                    [vN.ap[0], [H * HD1, SKC], [HD1, H], [1, 1]]),
            1.0,
        )

        # ---------------- phase A-C: stage x, cast weights, projections ----
        with ExitStack() as abc:
            stage = abc.enter_context(tc.tile_pool(name="stage", bufs=2))
            xtp = abc.enter_context(tc.tile_pool(name="xtp", bufs=1))
            pproj = abc.enter_context(
                tc.tile_pool(name="pproj", bufs=4, space="PSUM")
            )

            xT = xtp.tile([P, DCH, S], bf16)
            xTq = xtp.tile([P, DCH, SQ], bf16)

            # load 256-row groups, cast to bf16, transpose on the PE
            def stage_transpose(src, nrows, dstT):
                for g in range(nrows // 256):
                    xf = stage.tile([P, 2, D], f32, tag="ld")
                    nc.gpsimd.dma_start(
                        xf,
                        src[g * 256 : (g + 1) * 256, :].rearrange(
                            "(a p) d -> p a d", p=P
                        ),
                    )
                    xb = stage.tile([P, 2, D], bf16, tag="xb")
                    nc.vector.tensor_copy(xb, xf)
                    for a in range(2):
                        t = 2 * g + a
                        for half in range(2):
                            pst = psc.tile([P, SQ], bf16, tag="sc")
                            for jj in range(4):
                                j = 4 * half + jj
                                nc.tensor.transpose(
                                    pst[:, jj * P : (jj + 1) * P],
                                    xb[:, a, j * P : (j + 1) * P],
                                    identbf,
                                )
                            nc.vector.tensor_copy(
                                dstT[:, 4 * half : 4 * half + 4,
                                     t * P : (t + 1) * P],
                                pst.rearrange("p (j c) -> p j c", c=P),
                            )

            stage_transpose(xfull, S, xT)
            stage_transpose(xq, SQ, xTq)

            def load_w(pool, w):
                wb = pool.tile([P, DCH, D], bf16, name=f"wb_{w.name}")
                for g in range(DCH // 2):
                    wf = stage.tile([P, 2, D], f32, tag="ld")
                    nc.gpsimd.dma_start(
                        wf,
                        w[g * 256 : (g + 1) * 256, :].rearrange(
                            "(a p) d -> p a d", p=P
                        ),
                    )
                    nc.vector.tensor_copy(wb[:, 2 * g : 2 * g + 2, :], wf)
                return wb

            # --- Q projection: qT[dout, sq] = (Wq^T x^T + bq) / 8 ---
            with tc.tile_pool(name="wqp", bufs=1) as wqp:
                wqb = load_w(wqp, wq)
                for j in range(DCH):
                    ps = pproj.tile([P, SQ], f32, tag="pp")
                    for i in range(DCH):
                        nc.tensor.matmul(
                            ps,
                            lhsT=wqb[:, i, j * P : (j + 1) * P],
                            rhs=xTq[:, i, :],
                            start=(i == 0),
                            stop=(i == DCH - 1),
                        )
                    nc.scalar.activation(
                        qT[:, j, :], ps, Act.Identity,
                        bias=bqT8[:, j : j + 1], scale=1.0 / float(np.sqrt(HD)),
                    )

            # --- K projection: kT[dout, s] over the full batch ---
            with tc.tile_pool(name="wkp", bufs=1) as wkp:
                wkb = load_w(wkp, wk)
                for j in range(DCH):
                    pss = [pproj.tile([P, SQ], f32, tag="pp", name=f"psk{j}_{ti}") for ti in range(4)]
                    for i in range(DCH):
                        for t in range(S // SQ):
                            nc.tensor.matmul(
                                pss[t],
                                lhsT=wkb[:, i, j * P : (j + 1) * P],
                                rhs=xT[:, i, t * SQ : (t + 1) * SQ],
                                start=(i == 0),
                                stop=(i == DCH - 1),
                            )
                    for t in range(S // SQ):
                        nc.scalar.activation(
                            kT[:, j, t * SQ : (t + 1) * SQ], pss[t],
                            Act.Identity, bias=bkTt[:, j : j + 1],
                        )

            # --- V projection: natural layout [s, dout], full batch ---
            with tc.tile_pool(name="wvp", bufs=1) as wvp:
                wvb = load_w(wvp, wv)
                for t in range(SKC):
                    psn = [pproj.tile([P, SQ], f32, tag="pp", name=f"psv{t}_{ni}") for ni in range(2)]
                    for i in range(DCH):
                        for n in range(D // SQ):
                            nc.tensor.matmul(
                                psn[n],
                                lhsT=xT[:, i, t * P : (t + 1) * P],
                                rhs=wvb[:, i, n * SQ : (n + 1) * SQ],
                                start=(i == 0),
                                stop=(i == DCH - 1),
                            )
                    for n in range(D // SQ):
                        ps = psn[n]
                        nc.vector.tensor_copy(
                            bass.AP(
                                vN.tensor,
                                vN.offset + t * H * HD1 + n * 8 * HD1,
                                [vN.ap[0], [HD1, 8], [1, HD]],
                            ),
                            ps.rearrange("p (h c) -> p h c", c=HD),
                        )

        # ---------------- phases D-F ----------------
        persist2 = top.enter_context(tc.tile_pool(name="persist2", bufs=1))
        accT = persist2.tile([P, SKC, SQ], f32)  # sum_h attn_h^T / 16
        ctxT = persist2.tile([64, H, SQ], bf16)  # attention output, transposed

        # ---------------- phase D: attention, head pairs ----------------
        with ExitStack() as dph:
            ptp = dph.enter_context(tc.tile_pool(name="ptp", bufs=2))
            rp = dph.enter_context(tc.tile_pool(name="rp", bufs=2))
            ppv = dph.enter_context(tc.tile_pool(name="ppv", bufs=4, space="PSUM"))

            def emit_scores(pj, m):
                sc0 = psc.tile([P, SQ], f32, tag="sc", name=f"sc0_{pj}_{m}")
                sc1 = psc.tile([P, SQ], f32, tag="sc", name=f"sc1_{pj}_{m}")
                nc.tensor.matmul(
                    sc0,
                    lhsT=kT[0:64, pj, m * P : (m + 1) * P],
                    rhs=qT[0:64, pj, :],
                    start=True,
                    stop=True,
                )
                nc.tensor.matmul(
                    sc1,
                    lhsT=kT[64:128, pj, m * P : (m + 1) * P],
                    rhs=qT[64:128, pj, :],
                    start=True,
                    stop=True,
                )
                return sc0, sc1

            def emit_exp(pts, scs, m):
                nc.scalar.activation(pts[0][:, m, :], scs[0], Act.Exp)
                nc.scalar.activation(pts[1][:, m, :], scs[1], Act.Exp)

            def emit_pv(pj, pvs, pts, m):
                h0, h1 = 2 * pj, 2 * pj + 1
                st, sp = (m == 0), (m == SKC - 1)
                nc.tensor.matmul(
                    pvs[0][0:65, :],
                    lhsT=vN[:, m, h0 * HD1 : h0 * HD1 + HD1],
                    rhs=pts[0][:, m, :],
                    start=st,
                    stop=sp,
                    skip_group_check=True,
                )
                nc.tensor.matmul(
                    pvs[1][0:65, :],
                    lhsT=vN[:, m, h1 * HD1 : h1 * HD1 + HD1],
                    rhs=pts[1][:, m, :],
                    start=st,
                    stop=sp,
                    skip_group_check=True,
                )

            def make_tail(pj, pvs, pts):
                def tail():
                    for hh in range(2):
                        pv, pt = pvs[hh], pts[hh]
                        h = 2 * pj + hh
                        # r/H = exp(-ln(denom) - ln(H)) on the scalar engine
                        lnr = psc.tile([P, SQ], f32, tag="sc",
                                       name=f"lnr{pj}_{hh}")
                        nc.scalar.activation(
                            lnr[64:65, :], pv[64:65, :], Act.Ln
                        )
                        rrow = rp.tile([P, SQ], bf16, tag="rr", bufs=1,
                                       name=f"rrow{pj}_{hh}")
                        nc.scalar.activation(
                            rrow[64:65, :],
                            lnr[64:65, :],
                            Act.Exp,
                            bias=negln16[64:65, :],
                            scale=-1.0,
                        )
                        rbp = psc.tile([P, SQ], f32, tag="sc",
                                       name=f"rbp{pj}_{hh}")
                        nc.tensor.matmul(
                            rbp,
                            lhsT=ones_mat[64:65, :],
                            rhs=rrow[64:65, :],
                            start=True,
                            stop=True,
                        )
                        rbc = rp.tile([P, SQ], bf16, tag="rbc",
                                      name=f"rbc{pj}_{hh}")
                        nc.vector.tensor_copy(rbc, rbp)
                        # normalized ctx^T + v bias (attn weights sum to 1)
                        nc.vector.tensor_tensor(
                            ctxT[0:64, h, :], pv[0:64, :], rbc[0:64, :],
                            Alu.mult,
                        )
                        nc.vector.tensor_scalar(
                            ctxT[0:64, h, :], ctxT[0:64, h, :],
                            bvH[0:64, h : h + 1], None, Alu.add
                        )
                        # scale P~ by r/H in place for the mean accumulation
                        nc.vector.tensor_tensor(
                            pt, pt, free_bcast(rbc, SKC, SQ), Alu.mult
                        )
                    if pj == 0:
                        nc.vector.tensor_add(accT, pts[0], pts[1])
                    else:
                        nc.vector.tensor_add(pts[0], pts[0], pts[1])
                        nc.vector.tensor_add(accT, accT, pts[0])

                return tail

            tail_prev = None
            for pj in range(H // 2):
                pts = (
                    ptp.tile([P, SKC, SQ], bf16, tag="pt0", name=f"pt0_{pj}"),
                    ptp.tile([P, SKC, SQ], bf16, tag="pt1", name=f"pt1_{pj}"),
                )
                pvs = (
                    ppv.tile([P, SQ], f32, tag="pv", name=f"pv0_{pj}"),
                    ppv.tile([P, SQ], f32, tag="pv", name=f"pv1_{pj}"),
                )
                prev_sc = None
                for m in range(SKC):
                    scs = emit_scores(pj, m)
                    emit_exp(pts, scs, m)
                    if m == 4 and tail_prev is not None:
                        tail_prev()
                        tail_prev = None
                    if m >= 1:
                        emit_pv(pj, pvs, pts, m - 1)
                emit_pv(pj, pvs, pts, SKC - 1)
                tail_prev = make_tail(pj, pvs, pts)
            tail_prev()

        # ------- phases E+F interleaved: LayerNorm rows + attn-mean out ----
        with ExitStack() as eph:
            ep = eph.enter_context(tc.tile_pool(name="ep", bufs=2))
            gam_bc = ep.tile([P, D], f32, bufs=1)
            nc.gpsimd.dma_start(gam_bc, bcast_part(gamma.ap(), P))
            bet_bc = ep.tile([P, D], f32, bufs=1)
            nc.gpsimd.dma_start(bet_bc, bcast_part(beta.ap(), P))

            for t in range(SQT):
                # F: transpose the attn-mean accumulator block to natural
                ams = ep.tile([P, S], f32, tag="ams")
                for g in range(4):
                    pam = psc.tile([P, SQ], f32, tag="sc")
                    for u in range(4):
                        nc.tensor.transpose(
                            pam[:, u * P : (u + 1) * P],
                            accT[:, 4 * g + u, t * P : (t + 1) * P],
                            ident32,
                        )
                    nc.scalar.copy(ams[:, g * SQ : (g + 1) * SQ], pam)
                nc.gpsimd.dma_start(am_r[t * P : (t + 1) * P, :], ams)

                # E: ctx transpose + residual + LayerNorm
                pctx = psc.tile([P, DCH * P], bf16, tag="sc")
                for h in range(H):
                    nc.tensor.transpose(
                        pctx[:, h * HD : (h + 1) * HD],
                        ctxT[0:64, h, t * P : (t + 1) * P],
                        identbf[0:64, 0:64],
                    )
                xqs = ep.tile([P, D], f32, tag="xqs")
                nc.gpsimd.dma_start(xqs, xq[t * P : (t + 1) * P, :])
                y = ep.tile([P, D], f32, tag="y")
                # ctx was scaled by r/H; restore the factor H here
                nc.vector.scalar_tensor_tensor(
                    y, pctx, float(H), xqs, Alu.mult, Alu.add
                )
                stats = ep.tile([P, 2, 6], f32, tag="stats")
                nc.vector.bn_stats(stats[:, 0, :], y[:, 0:512])
                nc.vector.bn_stats(stats[:, 1, :], y[:, 512:1024])
                mv = ep.tile([P, 2], f32, tag="mv")
                nc.vector.bn_aggr(mv, stats)
                sd = ep.tile([P, 1], f32, tag="sd")
                nc.scalar.activation(
                    sd, mv[:, 1:2], Act.Sqrt, bias=epst, scale=1.0
                )
                rs = ep.tile([P, 1], f32, tag="rs")
                nc.vector.reciprocal(rs, sd)
                t1 = ep.tile([P, D], f32, tag="t1")
                nc.vector.tensor_scalar(
                    t1, y, mv[:, 0:1], rs, Alu.subtract, Alu.mult
                )
                t2 = ep.tile([P, D], f32, tag="t2")
                nc.vector.tensor_mul(t2, t1, gam_bc)
                nc.vector.tensor_add(t2, t2, bet_bc)
                nc.gpsimd.dma_start(out_r[t * P : (t + 1) * P, :], t2)

    _split_dma_waits(nc)
    return nc


def _split_dma_waits(nc):
    """walrus' dynamic-DMA (PSEUDO_DMA_DIRECT2D) lowering accepts at most one
    sync wait per DMA instruction, but Tile can attach several (its
    redundant-wait elision is disabled). Move all waits of a multi-wait DMA
    onto a NoOp inserted just before it on the same engine queue — the
    sequencer executes in order, so the gating is identical."""
    import concourse.mybir as mybir

    import concourse.bass_isa as bass_isa

    for bb in nc.m.functions[0].blocks:
        new_insts = []
        for inst in bb.instructions:
            si = getattr(inst, "sync_info", None)
            # raw-encoded ISA instructions (custom DVE ops) can carry no waits
            keep = 0 if isinstance(inst, bass_isa.InstCustomDveAnt) else 1
            if si is not None and si.on_wait and len(si.on_wait) > keep:
                moved = si.on_wait if keep == 0 else si.on_wait[:-1]
                kept = [] if keep == 0 else [si.on_wait[-1]]
                for wi, w in enumerate(moved):
                    new_insts.append(
                        mybir.InstNoOp(
                            name=f"{inst.name}_w{wi}",
                            ins=[],
                            outs=[],
                            engine=inst.engine,
                            sync_info=mybir.SyncInfo(on_wait=[w], on_update=[]),
                        )
                    )
                inst.sync_info = mybir.SyncInfo(
                    on_wait=kept, on_update=si.on_update
                )
            new_insts.append(inst)
        bb.instructions[:] = new_insts


def _get_nc():
    if "nc" not in _CACHE:
        _CACHE["nc"] = _build_nc()
    return _CACHE["nc"]


def kernel(x, Wq, bq, Wk, bk, Wv, bv, gamma, beta):
    from concourse.bass_utils import run_bass_kernel_spmd

    trace = bool(int(os.environ.get("KERNEL_TRACE", "0")))
    if trace:
        _ensure_ntff_hook()
        import concourse.bass_utils as _bu

        _bu.upload_artifacts = lambda d: str(d)  # no S3 in this container

    x = np.ascontiguousarray(np.asarray(x, dtype=np.float32))
    Wq = np.ascontiguousarray(np.asarray(Wq, dtype=np.float32))
    Wk = np.ascontiguousarray(np.asarray(Wk, dtype=np.float32))
    Wv = np.ascontiguousarray(np.asarray(Wv, dtype=np.float32))
    bq = np.ascontiguousarray(np.asarray(bq, dtype=np.float32))
    bk = np.ascontiguousarray(np.asarray(bk, dtype=np.float32))
    bv = np.ascontiguousarray(np.asarray(bv, dtype=np.float32))
    gamma = np.ascontiguousarray(np.asarray(gamma, dtype=np.float32))
    beta = np.ascontiguousarray(np.asarray(beta, dtype=np.float32))

    nc = _get_nc()
    in_maps = []
    for c in range(NCORES):
        b, g = c // GROUPS, c % GROUPS
        in_maps.append(
            {
                "xfull": x[b],
                "xq": np.ascontiguousarray(x[b, g * SQ : (g + 1) * SQ]),
                "wq": Wq,
                "wk": Wk,
                "wv": Wv,
                "bq": bq,
                "bk": bk,
                "bv": bv,
                "gamma": gamma,
                "beta": beta,
            }
        )

    res = run_bass_kernel_spmd(
        nc,
        in_maps,
        core_ids=list(range(NCORES)),
        trace=trace,
    )
    _CACHE["last_result"] = res

    out = np.empty((B, S, D), dtype=np.float32)
    am = np.empty((B, S, S), dtype=np.float32)
    for c in range(NCORES):
        b, g = c // GROUPS, c % GROUPS
        out[b, g * SQ : (g + 1) * SQ] = res.results[c]["out_rows"]
        am[b, g * SQ : (g + 1) * SQ] = res.results[c]["am_rows"]
    return out, am


# revision 37
# speedup vs baseline: 1.2076x; 1.2076x over previous
"""Trainium2 Bass kernel for nn_AttentionLayer: attention + residual LayerNorm.

Reference (per batch b):
    q/k/v = x @ W* + b*             [S, D], heads H=16 x HD=64
    attn  = softmax(q k^T / 8)      per head
    ctx   = attn @ v
    out   = LayerNorm(x + ctx) * gamma + beta
    am    = attn.mean(heads)

Sharding: 8 cores = 2 batches x 4 row-blocks. Core c handles batch c//4 and
query rows [512*(c%4), 512*(c%4+1)).  K/V are computed for the full batch on
every core (duplicated, no collectives); attention/softmax/mean/LN are fully
local to the core's 512 query rows.

Formulation: scores are computed TRANSPOSED (sk on partitions, sq on free),
softmax denominators via ones-matmul on the PE (broadcast-replicated across
partitions), so P~ = exp(scores^T) feeds the PV matmul directly from SBUF with
no transposes of the big attention matrix.
"""

import os

os.environ.pop("JAX_PLATFORMS", None)

import numpy as np

B, S, D, H, HD = 2, 2048, 1024, 16, 64
NCORES = 8
GROUPS = 4  # cores per batch
SQ = S // GROUPS  # 512 query rows per core
P = 128
DCH = D // P  # 8 d-chunks
SKC = S // P  # 16 sk-chunks
SQT = SQ // P  # 4 sq row blocks per core
EPS = 1e-5

_CACHE = {}


def _ensure_ntff_hook():
    """Restore the axon NTFF profiling hook if the image lacks
    antenv.axon_hooks (needed only when tracing; harmless otherwise)."""
    import sys
    import types

    try:
        from antenv.axon_hooks import get_axon_ntff_profile_hook  # noqa: F401

        return
    except ImportError:
        pass
    try:
        import antenv
        from trn_agent_boot.trn_boot import _ntff_profile_via_ctypes

        so = "/opt/axon/libaxon_pjrt.so"
        if not os.path.exists(so):
            return
        hook = _ntff_profile_via_ctypes(so)
        mod = types.ModuleType("antenv.axon_hooks")
        state = {"hook": hook}
        mod.set_axon_ntff_profile_hook = lambda h: state.__setitem__("hook", h)
        mod.get_axon_ntff_profile_hook = lambda: state["hook"]
        sys.modules["antenv.axon_hooks"] = mod
        antenv.axon_hooks = mod
    except Exception:
        pass


def _build_nc():
    from contextlib import ExitStack

    import concourse.bass as bass
    import concourse.mybir as mybir
    import concourse.tile as tile
    from concourse.masks import make_identity

    f32 = mybir.dt.float32
    bf16 = mybir.dt.bfloat16
    Alu = mybir.AluOpType
    Act = mybir.ActivationFunctionType

    nc = bass.Bass(target_bir_lowering=False, trn_type="TRN2")

    xfull = nc.dram_tensor("xfull", [S, D], f32, kind="ExternalInput")
    xq = nc.dram_tensor("xq", [SQ, D], f32, kind="ExternalInput")
    wq = nc.dram_tensor("wq", [D, D], f32, kind="ExternalInput")
    wk = nc.dram_tensor("wk", [D, D], f32, kind="ExternalInput")
    wv = nc.dram_tensor("wv", [D, D], f32, kind="ExternalInput")
    bq = nc.dram_tensor("bq", [D], f32, kind="ExternalInput")
    bk = nc.dram_tensor("bk", [D], f32, kind="ExternalInput")
    bv = nc.dram_tensor("bv", [D], f32, kind="ExternalInput")
    gamma = nc.dram_tensor("gamma", [D], f32, kind="ExternalInput")
    beta = nc.dram_tensor("beta", [D], f32, kind="ExternalInput")
    out_r = nc.dram_tensor("out_rows", [SQ, D], f32, kind="ExternalOutput")
    am_r = nc.dram_tensor("am_rows", [SQ, S], f32, kind="ExternalOutput")

    def bcast_part(ap, n):
        # replicate a [1, F]-ish AP across n partitions (DRAM source only)
        return bass.AP(ap.tensor, ap.offset, [[0, n]] + list(ap.ap))

    def free_bcast(ap, reps, inner):
        # [P, inner] -> [P, reps, inner] with 0-stride middle dim
        a = list(ap.ap)
        return bass.AP(ap.tensor, ap.offset, [a[0], [0, reps], a[-1]])

    with tile.TileContext(nc) as tc, ExitStack() as top:
        const = top.enter_context(tc.tile_pool(name="const", bufs=1))
        persist = top.enter_context(tc.tile_pool(name="persist", bufs=1))
        psc = top.enter_context(tc.tile_pool(name="psc", bufs=4, space="PSUM"))

        ident32 = const.tile([P, P], f32)
        make_identity(nc, ident32)
        identbf = const.tile([P, P], bf16)
        make_identity(nc, identbf)
        ones_mat = const.tile([P, P], bf16)
        nc.vector.memset(ones_mat, 1.0)
        epst = const.tile([P, 1], f32)
        nc.vector.memset(epst, EPS)
        negln16 = const.tile([P, 1], f32)
        nc.vector.memset(negln16, -float(np.log(H)))
        # biases transposed onto partitions: [128, DCH] (d = j*128 + p) and
        # per-head [64, H] for the v bias
        bkTt = const.tile([P, DCH], f32)
        nc.gpsimd.dma_start(bkTt, bass.AP(bk, 0, [[1, P], [P, DCH]]))
        bqT8 = const.tile([P, DCH], f32)
        nc.gpsimd.dma_start(bqT8, bass.AP(bq, 0, [[1, P], [P, DCH]]))
        nc.vector.tensor_scalar_mul(bqT8, bqT8, 1.0 / float(np.sqrt(HD)))
        bvH = const.tile([64, H], f32)
        nc.gpsimd.dma_start(bvH, bass.AP(bv, 0, [[1, 64], [64, H]]))
        nc.vector.tensor_scalar_mul(bvH, bvH, 1.0 / H)
        # warm the PE clock gate with a short dense burst
        for _ in range(16):
            wps = psc.tile([P, SQ], f32, tag="sc")
            nc.tensor.matmul(
                wps, lhsT=ones_mat, rhs=free_bcast(ones_mat, 4, P),
                start=True, stop=True,
            )

        # big persistent tensors (live to the end)
        kT = persist.tile([P, DCH, S], bf16)  # k^T: [dout, s]
        # v: [s, per-head (64 cols + a ones column)] chunked by s
        HD1 = HD + 1
        vN = persist.tile([P, SKC, H * HD1], bf16)
        qT = persist.tile([P, DCH, SQ], bf16)  # (q/8)^T for this core's rows
        nc.vector.memset(
            bass.AP(vN.tensor, vN.offset + HD,
                    [vN.ap[0], [H * HD1, SKC], [HD1, H], [1, 1]]),
            1.0,
        )

        # ---------------- phase A-C: stage x, cast weights, projections ----
        with ExitStack() as abc:
            stage = abc.enter_context(tc.tile_pool(name="stage", bufs=2))
            xtp = abc.enter_context(tc.tile_pool(name="xtp", bufs=1))
            pproj = abc.enter_context(
                tc.tile_pool(name="pproj", bufs=4, space="PSUM")
            )

            xT = xtp.tile([P, DCH, S], bf16)
            xTq = xtp.tile([P, DCH, SQ], bf16)

            # load 256-row groups, cast to bf16, transpose on the PE
            def stage_transpose(src, nrows, dstT):
                for g in range(nrows // 256):
                    xf = stage.tile([P, 2, D], f32, tag="ld")
                    nc.gpsimd.dma_start(
                        xf,
                        src[g * 256 : (g + 1) * 256, :].rearrange(
                            "(a p) d -> p a d", p=P
                        ),
                    )
                    xb = stage.tile([P, 2, D], bf16, tag="xb")
                    nc.vector.tensor_copy(xb, xf)
                    for a in range(2):
                        t = 2 * g + a
                        for half in range(2):
                            pst = psc.tile([P, SQ], bf16, tag="sc")
                            for jj in range(4):
                                j = 4 * half + jj
                                nc.tensor.transpose(
                                    pst[:, jj * P : (jj + 1) * P],
                                    xb[:, a, j * P : (j + 1) * P],
                                    identbf,
                                )
                            nc.vector.tensor_copy(
                                dstT[:, 4 * half : 4 * half + 4,
                                     t * P : (t + 1) * P],
                                pst.rearrange("p (j c) -> p j c", c=P),
                            )

            stage_transpose(xfull, S, xT)
            stage_transpose(xq, SQ, xTq)

            def load_w(pool, w):
                wb = pool.tile([P, DCH, D], bf16, name=f"wb_{w.name}")
                for g in range(DCH // 2):
                    wf = stage.tile([P, 2, D], f32, tag="ld")
                    nc.gpsimd.dma_start(
                        wf,
                        w[g * 256 : (g + 1) * 256, :].rearrange(
                            "(a p) d -> p a d", p=P
                        ),
                    )
                    nc.vector.tensor_copy(wb[:, 2 * g : 2 * g + 2, :], wf)
                return wb

            # --- Q projection: qT[dout, sq] = (Wq^T x^T + bq) / 8 ---
            with tc.tile_pool(name="wqp", bufs=1) as wqp:
                wqb = load_w(wqp, wq)
                for j in range(DCH):
                    ps = pproj.tile([P, SQ], f32, tag="pp")
                    for i in range(DCH):
                        nc.tensor.matmul(
                            ps,
                            lhsT=wqb[:, i, j * P : (j + 1) * P],
                            rhs=xTq[:, i, :],
                            start=(i == 0),
                            stop=(i == DCH - 1),
                        )
                    nc.scalar.activation(
                        qT[:, j, :], ps, Act.Identity,
                        bias=bqT8[:, j : j + 1], scale=1.0 / float(np.sqrt(HD)),
                    )

            # --- K projection: kT[dout, s] over the full batch ---
            with tc.tile_pool(name="wkp", bufs=1) as wkp:
                wkb = load_w(wkp, wk)
                for j in range(DCH):
                    pss = [pproj.tile([P, SQ], f32, tag="pp", name=f"psk{j}_{ti}") for ti in range(4)]
                    for i in range(DCH):
                        for t in range(S // SQ):
                            nc.tensor.matmul(
                                pss[t],
                                lhsT=wkb[:, i, j * P : (j + 1) * P],
                                rhs=xT[:, i, t * SQ : (t + 1) * SQ],
                                start=(i == 0),
                                stop=(i == DCH - 1),
                            )
                    for t in range(S // SQ):
                        nc.scalar.activation(
                            kT[:, j, t * SQ : (t + 1) * SQ], pss[t],
                            Act.Identity, bias=bkTt[:, j : j + 1],
                        )

            # --- V projection: natural layout [s, dout], full batch ---
            with tc.tile_pool(name="wvp", bufs=1) as wvp:
                wvb = load_w(wvp, wv)
                for t in range(SKC):
                    psn = [pproj.tile([P, SQ], f32, tag="pp", name=f"psv{t}_{ni}") for ni in range(2)]
                    for i in range(DCH):
                        for n in range(D // SQ):
                            nc.tensor.matmul(
                                psn[n],
                                lhsT=xT[:, i, t * P : (t + 1) * P],
                                rhs=wvb[:, i, n * SQ : (n + 1) * SQ],
                                start=(i == 0),
                                stop=(i == DCH - 1),
                            )
                    for n in range(D // SQ):
                        ps = psn[n]
                        nc.vector.tensor_copy(
                            bass.AP(
                                vN.tensor,
                                vN.offset + t * H * HD1 + n * 8 * HD1,
                                [vN.ap[0], [HD1, 8], [1, HD]],
                            ),
                            ps.rearrange("p (h c) -> p h c", c=HD),
                        )

        # ---------------- phases D-F ----------------
        persist2 = top.enter_context(tc.tile_pool(name="persist2", bufs=1))
        accT = persist2.tile([P, SKC, SQ], f32)  # sum_h attn_h^T / 16
        ctxT = persist2.tile([64, H, SQ], bf16)  # attention output, transposed

        # ---------------- phase D: attention, head pairs ----------------
        with ExitStack() as dph:
            ptp = dph.enter_context(tc.tile_pool(name="ptp", bufs=2))
            rp = dph.enter_context(tc.tile_pool(name="rp", bufs=2))
            ppv = dph.enter_context(tc.tile_pool(name="ppv", bufs=4, space="PSUM"))

            def emit_scores(pj, m):
                sc0 = psc.tile([P, SQ], f32, tag="sc", name=f"sc0_{pj}_{m}")
                sc1 = psc.tile([P, SQ], f32, tag="sc", name=f"sc1_{pj}_{m}")
                nc.tensor.matmul(
                    sc0,
                    lhsT=kT[0:64, pj, m * P : (m + 1) * P],
                    rhs=qT[0:64, pj, :],
                    start=True,
                    stop=True,
                )
                nc.tensor.matmul(
                    sc1,
                    lhsT=kT[64:128, pj, m * P : (m + 1) * P],
                    rhs=qT[64:128, pj, :],
                    start=True,
                    stop=True,
                )
                return sc0, sc1

            def emit_exp(pts, scs, m):
                nc.scalar.activation(pts[0][:, m, :], scs[0], Act.Exp)
                nc.scalar.activation(pts[1][:, m, :], scs[1], Act.Exp)

            def emit_pv(pj, pvs, pts, m):
                h0, h1 = 2 * pj, 2 * pj + 1
                st, sp = (m == 0), (m == SKC - 1)
                nc.tensor.matmul(
                    pvs[0][0:65, :],
                    lhsT=vN[:, m, h0 * HD1 : h0 * HD1 + HD1],
                    rhs=pts[0][:, m, :],
                    start=st,
                    stop=sp,
                    skip_group_check=True,
                )
                nc.tensor.matmul(
                    pvs[1][0:65, :],
                    lhsT=vN[:, m, h1 * HD1 : h1 * HD1 + HD1],
                    rhs=pts[1][:, m, :],
                    start=st,
                    stop=sp,
                    skip_group_check=True,
                )

            def emit_tail_act(pj, pvs):
                # r/H = exp(-ln(denom) - ln(H)) on the scalar engine; emitted
                # immediately after the pair so it beats the next pair's exps
                # into the ACT queue
                rrows = []
                for hh in range(2):
                    pv = pvs[hh]
                    lnr = psc.tile([P, SQ], f32, tag="sc",
                                   name=f"lnr{pj}_{hh}")
                    nc.scalar.activation(
                        lnr[64:65, :], pv[64:65, :], Act.Ln
                    )
                    rrow = rp.tile([P, SQ], bf16, tag="rr", bufs=2,
                                   name=f"rrow{pj}_{hh}")
                    nc.scalar.activation(
                        rrow[64:65, :],
                        lnr[64:65, :],
                        Act.Exp,
                        bias=negln16[64:65, :],
                        scale=-1.0,
                    )
                    rrows.append(rrow)
                return rrows

            def make_tail(pj, pvs, pts, rrows):
                def tail():
                    for hh in range(2):
                        pv, pt = pvs[hh], pts[hh]
                        h = 2 * pj + hh
                        rbp = psc.tile([P, SQ], f32, tag="sc",
                                       name=f"rbp{pj}_{hh}")
                        nc.tensor.matmul(
                            rbp,
                            lhsT=ones_mat[64:65, :],
                            rhs=rrows[hh][64:65, :],
                            start=True,
                            stop=True,
                        )
                        rbc = rp.tile([P, SQ], bf16, tag="rbc",
                                      name=f"rbc{pj}_{hh}")
                        nc.vector.tensor_copy(rbc, rbp)
                        # normalized ctx^T + v bias (attn weights sum to 1)
                        nc.vector.tensor_tensor(
                            ctxT[0:64, h, :], pv[0:64, :], rbc[0:64, :],
                            Alu.mult,
                        )
                        nc.vector.tensor_scalar(
                            ctxT[0:64, h, :], ctxT[0:64, h, :],
                            bvH[0:64, h : h + 1], None, Alu.add
                        )
                        # scale P~ by r/H in place for the mean accumulation
                        nc.vector.tensor_tensor(
                            pt, pt, free_bcast(rbc, SKC, SQ), Alu.mult
                        )
                    if pj == 0:
                        nc.vector.tensor_add(accT, pts[0], pts[1])
                    else:
                        nc.vector.tensor_add(pts[0], pts[0], pts[1])
                        nc.vector.tensor_add(accT, accT, pts[0])

                return tail

            tail_prev = None
            for pj in range(H // 2):
                pts = (
                    ptp.tile([P, SKC, SQ], bf16, tag="pt0", name=f"pt0_{pj}"),
                    ptp.tile([P, SKC, SQ], bf16, tag="pt1", name=f"pt1_{pj}"),
                )
                pvs = (
                    ppv.tile([P, SQ], f32, tag="pv", name=f"pv0_{pj}"),
                    ppv.tile([P, SQ], f32, tag="pv", name=f"pv1_{pj}"),
                )
                prev_sc = None
                for m in range(SKC):
                    scs = emit_scores(pj, m)
                    emit_exp(pts, scs, m)
                    if m == 2 and tail_prev is not None:
                        tail_prev()
                        tail_prev = None
                    if m >= 1:
                        emit_pv(pj, pvs, pts, m - 1)
                emit_pv(pj, pvs, pts, SKC - 1)
                rrows = emit_tail_act(pj, pvs)
                tail_prev = make_tail(pj, pvs, pts, rrows)
            tail_prev()

        # ------- phases E+F interleaved: LayerNorm rows + attn-mean out ----
        with ExitStack() as eph:
            ep = eph.enter_context(tc.tile_pool(name="ep", bufs=2))
            gam_bc = ep.tile([P, D], f32, bufs=1)
            nc.gpsimd.dma_start(gam_bc, bcast_part(gamma.ap(), P))
            bet_bc = ep.tile([P, D], f32, bufs=1)
            nc.gpsimd.dma_start(bet_bc, bcast_part(beta.ap(), P))

            for t in range(SQT):
                # F: transpose the attn-mean accumulator block to natural
                ams = ep.tile([P, S], f32, tag="ams")
                for g in range(4):
                    pam = psc.tile([P, SQ], f32, tag="sc")
                    for u in range(4):
                        nc.tensor.transpose(
                            pam[:, u * P : (u + 1) * P],
                            accT[:, 4 * g + u, t * P : (t + 1) * P],
                            ident32,
                        )
                    nc.scalar.copy(ams[:, g * SQ : (g + 1) * SQ], pam)
                nc.gpsimd.dma_start(am_r[t * P : (t + 1) * P, :], ams)

                # E: ctx transpose + residual + LayerNorm
                pctx = psc.tile([P, DCH * P], bf16, tag="sc")
                for h in range(H):
                    nc.tensor.transpose(
                        pctx[:, h * HD : (h + 1) * HD],
                        ctxT[0:64, h, t * P : (t + 1) * P],
                        identbf[0:64, 0:64],
                    )
                xqs = ep.tile([P, D], f32, tag="xqs")
                nc.gpsimd.dma_start(xqs, xq[t * P : (t + 1) * P, :])
                y = ep.tile([P, D], f32, tag="y")
                # ctx was scaled by r/H; restore the factor H here
                nc.vector.scalar_tensor_tensor(
                    y, pctx, float(H), xqs, Alu.mult, Alu.add
                )
                stats = ep.tile([P, 2, 6], f32, tag="stats")
                nc.vector.bn_stats(stats[:, 0, :], y[:, 0:512])
                nc.vector.bn_stats(stats[:, 1, :], y[:, 512:1024])
                mv = ep.tile([P, 2], f32, tag="mv")
                nc.vector.bn_aggr(mv, stats)
                sd = ep.tile([P, 1], f32, tag="sd")
                nc.scalar.activation(
                    sd, mv[:, 1:2], Act.Sqrt, bias=epst, scale=1.0
                )
                rs = ep.tile([P, 1], f32, tag="rs")
                nc.vector.reciprocal(rs, sd)
                t1 = ep.tile([P, D], f32, tag="t1")
                nc.vector.tensor_scalar(
                    t1, y, mv[:, 0:1], rs, Alu.subtract, Alu.mult
                )
                t2 = ep.tile([P, D], f32, tag="t2")
                nc.vector.tensor_mul(t2, t1, gam_bc)
                nc.vector.tensor_add(t2, t2, bet_bc)
                nc.gpsimd.dma_start(out_r[t * P : (t + 1) * P, :], t2)

    _split_dma_waits(nc)
    return nc


def _split_dma_waits(nc):
    """walrus' dynamic-DMA (PSEUDO_DMA_DIRECT2D) lowering accepts at most one
    sync wait per DMA instruction, but Tile can attach several (its
    redundant-wait elision is disabled). Move all waits of a multi-wait DMA
    onto a NoOp inserted just before it on the same engine queue — the
    sequencer executes in order, so the gating is identical."""
    import concourse.mybir as mybir

    import concourse.bass_isa as bass_isa

    for bb in nc.m.functions[0].blocks:
        new_insts = []
        for inst in bb.instructions:
            si = getattr(inst, "sync_info", None)
            # raw-encoded ISA instructions (custom DVE ops) can carry no waits
            keep = 0 if isinstance(inst, bass_isa.InstCustomDveAnt) else 1
            if si is not None and si.on_wait and len(si.on_wait) > keep:
                moved = si.on_wait if keep == 0 else si.on_wait[:-1]
                kept = [] if keep == 0 else [si.on_wait[-1]]
                for wi, w in enumerate(moved):
                    new_insts.append(
                        mybir.InstNoOp(
                            name=f"{inst.name}_w{wi}",
                            ins=[],
                            outs=[],
                            engine=inst.engine,
                            sync_info=mybir.SyncInfo(on_wait=[w], on_update=[]),
                        )
                    )
                inst.sync_info = mybir.SyncInfo(
                    on_wait=kept, on_update=si.on_update
                )
            new_insts.append(inst)
        bb.instructions[:] = new_insts


def _get_nc():
    if "nc" not in _CACHE:
        _CACHE["nc"] = _build_nc()
    return _CACHE["nc"]


def kernel(x, Wq, bq, Wk, bk, Wv, bv, gamma, beta):
    from concourse.bass_utils import run_bass_kernel_spmd

    trace = bool(int(os.environ.get("KERNEL_TRACE", "0")))
    if trace:
        _ensure_ntff_hook()
        import concourse.bass_utils as _bu

        _bu.upload_artifacts = lambda d: str(d)  # no S3 in this container

    x = np.ascontiguousarray(np.asarray(x, dtype=np.float32))
    Wq = np.ascontiguousarray(np.asarray(Wq, dtype=np.float32))
    Wk = np.ascontiguousarray(np.asarray(Wk, dtype=np.float32))
    Wv = np.ascontiguousarray(np.asarray(Wv, dtype=np.float32))
    bq = np.ascontiguousarray(np.asarray(bq, dtype=np.float32))
    bk = np.ascontiguousarray(np.asarray(bk, dtype=np.float32))
    bv = np.ascontiguousarray(np.asarray(bv, dtype=np.float32))
    gamma = np.ascontiguousarray(np.asarray(gamma, dtype=np.float32))
    beta = np.ascontiguousarray(np.asarray(beta, dtype=np.float32))

    nc = _get_nc()
    in_maps = []
    for c in range(NCORES):
        b, g = c // GROUPS, c % GROUPS
        in_maps.append(
            {
                "xfull": x[b],
                "xq": np.ascontiguousarray(x[b, g * SQ : (g + 1) * SQ]),
                "wq": Wq,
                "wk": Wk,
                "wv": Wv,
                "bq": bq,
                "bk": bk,
                "bv": bv,
                "gamma": gamma,
                "beta": beta,
            }
        )

    res = run_bass_kernel_spmd(
        nc,
        in_maps,
        core_ids=list(range(NCORES)),
        trace=trace,
    )
    _CACHE["last_result"] = res

    out = np.empty((B, S, D), dtype=np.float32)
    am = np.empty((B, S, S), dtype=np.float32)
    for c in range(NCORES):
        b, g = c // GROUPS, c % GROUPS
        out[b, g * SQ : (g + 1) * SQ] = res.results[c]["out_rows"]
        am[b, g * SQ : (g + 1) * SQ] = res.results[c]["am_rows"]
    return out, am
